# revision 22
# baseline (speedup 1.0000x reference)
"""Trainium2 Bass kernel for nn_BaselineModel_80796924772520 (dense_cnn).

Self-contained: kernel(**inputs) -> np.ndarray [512, 7] float32.

Strategy: pure data parallelism over 8 NeuronCores (64 images each).
 - BN folded into conv weights/biases on host (eval-mode BN is affine).
 - fc1/fc2/att collapse into one linear map W_eff [64, 2304] on host
   (reference has no nonlinearity between them).
 - All matmuls run as float32r (fast fp32 PE mode, 1 cycle/row at N>=256).
 - conv1 (C_in=1): im2col K=9 built by DMA from a zero-padded x copy.
 - conv2/conv3: 9-tap shifted-window accumulating matmuls over zero-padded
   SBUF activations (no im2col materialization).
 - maxpool: strided tensor_max pairs, pool applied to raw PSUM before
   bias+relu (max commutes with per-channel bias add; relu is monotone).
 - attention: per-image [64x36]^T@[64x1] matmuls -> PE transpose ->
   softmax -> broadcast-matmul with ones -> multiply+segmented reduce.
"""
import sys
if '/opt/trn_rl_repo' not in sys.path:
    sys.path.insert(0, '/opt/trn_rl_repo')

import contextlib
import numpy as np

import concourse.bass as bass
import concourse.mybir as mybir
import concourse.tile as tile
from concourse.masks import make_identity

F32 = mybir.dt.float32
F32R = mybir.dt.float32r
RELU = mybir.ActivationFunctionType.Relu
EXP = mybir.ActivationFunctionType.Exp

N_CORES = 8
B_TOTAL = 512
BPC = B_TOTAL // N_CORES   # 64 images per core
G = 8                      # images per group
NG = BPC // G              # 8 groups
EPS = 1e-5

_MAX_WAITS = 1  # this walrus build supports 1 sync-wait per instruction


def _install_tile_fixups():
    """The nix walrus here allows only ONE sync-wait per instruction; Tile's
    exit drain aggregates one wait per live proc onto a single Drain. Spread
    the waits across spare SP nops emitted just before the drain."""
    if getattr(tile.TileContext, '_drain_patched', False):
        return

    def _patched(self, tick_clock, wait_clock):
        from concourse.vector_clock import ScopedClock
        nc = self.nc
        nops = [nc.sync.nop().ins for _ in range(32)]
        drain_inst = nc.sync.drain()
        wait_clock.add_sem_waits(
            drain_inst.ins, ScopedClock({None: tick_clock.global_clock}))
        si = drain_inst.ins.sync_info
        if si is not None and len(si.on_wait) > _MAX_WAITS:
            waits = list(si.on_wait)
            drain_inst.ins.sync_info = mybir.SyncInfo(
                on_wait=waits[:_MAX_WAITS], on_update=list(si.on_update))
            rest = waits[_MAX_WAITS:]
            for i in range(0, len(rest), _MAX_WAITS):
                nops[i // _MAX_WAITS].sync_info = mybir.SyncInfo(
                    on_wait=rest[i:i + _MAX_WAITS], on_update=[])
        nc.all_engine_barrier()
        popped = nc._tile_sem_poison_stack.pop()
        assert popped is self._sem_poison
        nc.clear_and_free_semaphores(list(self.sems.allocated().values()))
        nc.all_engine_barrier()

    tile.TileContext._drain_and_barrier = _patched
    tile.TileContext._drain_patched = True


def _split_excess_waits(nc):
    """This walrus allows one sync-wait per instruction. Hoist excess waits
    onto same-engine nops inserted immediately before the instruction
    (sequential waits on one engine are equivalent to a combined wait)."""
    idx = 0
    for f in nc.m.functions:
        for b in f.blocks:
            out, changed = [], False
            for ins in b.instructions:
                si = ins.sync_info
                if si is not None and len(si.on_wait) > _MAX_WAITS:
                    waits = list(si.on_wait)
                    extra, keep = waits[:-_MAX_WAITS], waits[-_MAX_WAITS:]
                    for j in range(0, len(extra), _MAX_WAITS):
                        nop = mybir.InstNoOp(name=f"I-wsplit-{idx}")
                        idx += 1
                        nop.engine = ins.engine
                        nop.sync_info = mybir.SyncInfo(
                            on_wait=extra[j:j + _MAX_WAITS], on_update=[])
                        nc.register_instruction(nop, overwrite=True)
                        out.append(nop)
                    ins.sync_info = mybir.SyncInfo(
                        on_wait=keep, on_update=list(si.on_update))
                    changed = True
                out.append(ins)
            if changed:
                b.instructions = out


def _prep_weights(p):
    """Fold BN, collapse FC chain, lay out weights for the device program."""
    def fold(w, b, g, be, m, v):
        inv = (g / np.sqrt(v + EPS)).astype(np.float32)
        wf = (w * inv[:, None, None, None]).astype(np.float32)
        bf = ((b - m) * inv + be).astype(np.float32)
        return wf, bf

    w1, b1 = fold(p['conv1_w'], p['conv1_b'], p['bn1_g'], p['bn1_b'], p['bn1_m'], p['bn1_v'])
    w2, b2 = fold(p['conv2_w'], p['conv2_b'], p['bn2_g'], p['bn2_b'], p['bn2_m'], p['bn2_v'])
    w3, b3 = fold(p['conv3_w'], p['conv3_b'], p['bn3_g'], p['bn3_b'], p['bn3_m'], p['bn3_v'])

    # conv1 lhsT [128, 256]: rows 32q+k (k = 3*ky+kx) = w1[c, 0, ky, kx],
    # replicated into all 4 row-quadrants (enables 32-row PE tiling later).
    W1T = np.zeros((128, 256), np.float32)
    w1f = w1.reshape(256, 9)                      # [c, k]
    for q in range(4):
        W1T[32 * q:32 * q + 9, :] = w1f.T
    # conv2 lhsT [128, 2304]: [p, t*256 + h*128 + m] = w2[m, 128h+p, t]
    W2T = np.ascontiguousarray(
        w2.reshape(128, 2, 128, 9).transpose(2, 3, 1, 0)  # [p, t, h, m]
    ).reshape(128, 2304)
    # conv3 lhsT [128, 576]: [p, t*64 + m] = w3[m, p, t]
    W3T = np.ascontiguousarray(
        w3.reshape(64, 128, 9).transpose(1, 2, 0)).reshape(128, 576)

    # FC chain collapse: q = out4 @ W_eff.T + b_eff
    fc1w, fc2w, attw = p['fc1_w'], p['fc2_w'], p['att_w']
    W_eff = (attw @ fc2w @ fc1w).astype(np.float32)          # [64, 2304]
    b_eff = (attw @ (fc2w @ p['fc1_b'] + p['fc2_b']) + p['att_b']).astype(np.float32)
    # WeT2 [64, 2304]: [c, hw*64 + m] = W_eff[m, c*36 + hw]
    WeT2 = np.ascontiguousarray(
        W_eff.reshape(64, 64, 36).transpose(1, 2, 0)).reshape(64, 2304)

    W3fT = np.ascontiguousarray(p['fc3_w'].T).astype(np.float32)  # [64, 7]
    fc3b_rep = np.broadcast_to(p['fc3_b'], (64, 7)).astype(np.float32).copy()

    b1c = np.ascontiguousarray(b1.reshape(2, 128).T)       # [128, 2]
    b2c = b2.reshape(128, 1).astype(np.float32)
    b3c = b3.reshape(64, 1).astype(np.float32)
    beffc = b_eff.reshape(64, 1).astype(np.float32)

    return dict(W1T=W1T, W2T=W2T, W3T=W3T, WeT2=WeT2, W3fT=W3fT,
                fc3b_rep=fc3b_rep, b1c=b1c, b2c=b2c, b3c=b3c, beffc=beffc,
                Z=np.zeros((1, 12544), np.float32),
                ONES=np.ones((1, 64), np.float32))


def build_program(debug=False):
    """Build the per-core SPMD Bass program. Returns nc."""
    _install_tile_fixups()
    nc = bass.Bass("TRN2", target_bir_lowering=False, debug=False)

    x = nc.declare_dram_parameter("x", [BPC, 2304], F32R, isOutput=False)
    W1T = nc.declare_dram_parameter("W1T", [128, 256], F32R, isOutput=False)
    W2T = nc.declare_dram_parameter("W2T", [128, 2304], F32R, isOutput=False)
    W3T = nc.declare_dram_parameter("W3T", [128, 576], F32R, isOutput=False)
    WeT2 = nc.declare_dram_parameter("WeT2", [64, 2304], F32R, isOutput=False)
    W3fT = nc.declare_dram_parameter("W3fT", [64, 7], F32R, isOutput=False)
    fc3b = nc.declare_dram_parameter("fc3b_rep", [64, 7], F32, isOutput=False)
    b1c = nc.declare_dram_parameter("b1c", [128, 2], F32, isOutput=False)
    b2c = nc.declare_dram_parameter("b2c", [128, 1], F32, isOutput=False)
    b3c = nc.declare_dram_parameter("b3c", [64, 1], F32, isOutput=False)
    beffc = nc.declare_dram_parameter("beffc", [64, 1], F32, isOutput=False)
    Z = nc.declare_dram_parameter("Z", [1, 12544], F32R, isOutput=False)
    ONES = nc.declare_dram_parameter("ONES", [1, 64], F32R, isOutput=False)
    out = nc.declare_dram_parameter("out", [BPC, 7], F32, isOutput=True)
    dbg = {}
    if debug:
        for nm, shp in [("dbg_act1_0", [128, G * 676]), ("dbg_act1_1", [128, G * 676]),
                        ("dbg_act2", [128, BPC * 196]), ("dbg_out3", [64, BPC * 36]),
                        ("dbg_q", [64, 64]), ("dbg_attn", [64, 36]),
                        ("dbg_gT", [64, 64]), ("dbg_sc", [36, 64])]:
            dbg[nm] = nc.declare_dram_parameter(nm, shp, F32, isOutput=True)

    r = lambda ap: ap                 # tiles are already F32R-typed

    with tile.TileContext(nc) as tc, contextlib.ExitStack() as ctx:
        wp = ctx.enter_context(tc.tile_pool(name="weights", bufs=1))
        ap_pool = ctx.enter_context(tc.tile_pool(name="acts", bufs=1))
        cp = ctx.enter_context(tc.tile_pool(name="im2col", bufs=2))
        t1p = ctx.enter_context(tc.tile_pool(name="t1", bufs=2))
        t2p = ctx.enter_context(tc.tile_pool(name="t2", bufs=2))
        e2p = ctx.enter_context(tc.tile_pool(name="ev2", bufs=3))
        e3p = ctx.enter_context(tc.tile_pool(name="ev3", bufs=3))

        # ---- load weights ----
        w1t = wp.tile([128, 256], F32R)
        nc.sync.dma_start(out=w1t[:], in_=W1T[:])
        w2t = wp.tile([128, 2304], F32R)
        nc.sync.dma_start(out=w2t[:], in_=W2T[:])
        w3t = wp.tile([128, 576], F32R)
        nc.sync.dma_start(out=w3t[:], in_=W3T[:])
        wet = wp.tile([64, 2304], F32R)
        nc.sync.dma_start(out=wet[:], in_=WeT2[:])
        w3f = wp.tile([64, 7], F32R)
        nc.sync.dma_start(out=w3f[:], in_=W3fT[:])
        fc3b_t = wp.tile([64, 7], F32)
        nc.sync.dma_start(out=fc3b_t[:], in_=fc3b[:])
        b1t = wp.tile([128, 2], F32)
        nc.sync.dma_start(out=b1t[:], in_=b1c[:])
        b2t = wp.tile([128, 1], F32)
        nc.sync.dma_start(out=b2t[:], in_=b2c[:])
        b3t = wp.tile([64, 1], F32)
        nc.sync.dma_start(out=b3t[:], in_=b3c[:])
        bet = wp.tile([64, 1], F32)
        nc.sync.dma_start(out=bet[:], in_=beffc[:])
        ident = wp.tile([128, 128], F32)
        make_identity(nc, ident[:])
        ones1 = wp.tile([1, 64], F32R)
        nc.sync.dma_start(out=ones1[:], in_=ONES[:])

        # ---- persistent activation buffers ----
        act1 = [ap_pool.tile([128, G * 676], F32R, tag=f"act1_{h}",
                             name=f"act1_{h}") for h in range(2)]
        act2 = ap_pool.tile([128, BPC * 196], F32R)
        out3 = ap_pool.tile([64, BPC * 36], F32R)
        xpad = ap_pool.tile([8, 2500], F32R)
        # f32r tensors cannot be Memset on this walrus: zero-fill the padded
        # buffers once via broadcast-DMA from a zeros input.
        for h in range(2):
            nc.gpsimd.dma_start(out=act1[h][:],
                                in_=Z[:, :G * 676].to_broadcast((128, G * 676)))
        nc.gpsimd.dma_start(out=act2[:], in_=Z[:].to_broadcast((128, 12544)))
        nc.gpsimd.dma_start(out=xpad[:], in_=Z[:, :2500].to_broadcast((8, 2500)))

        with contextlib.ExitStack() as cctx:
            ps1 = cctx.enter_context(tc.tile_pool(name="ps1", bufs=2, space="PSUM"))
            ps2 = cctx.enter_context(tc.tile_pool(name="ps2", bufs=3, space="PSUM"))

            def conv1_img(g, ci, imt):
                """conv1 for one image (index ci in group g) + pool + relu."""
                for h in range(2):
                    t1 = t1p.tile([128, 1152], F32, tag="t1", name="t1")
                    todd = t1p.tile([128, 1152], F32, tag="todd", name="todd")
                    # psum tiles: 1024 + 1024 + 256 cols of the 2304-col image
                    for n, (lo, sz) in enumerate(((0, 1024), (1024, 1024), (2048, 256))):
                        ps = ps1.tile([128, 1024], F32, tag="ps1", name="ps1")
                        for j in range(0, sz, 512):
                            w = min(512, sz - j)
                            nc.tensor.matmul(
                                out=ps[:, j:j + w],
                                lhsT=r(w1t[0:9, 128 * h:128 * (h + 1)]),
                                rhs=r(imt[0:9, lo + j:lo + j + w]),
                                start=True, stop=True)
                        # DVE may read only one PSUM operand: ACT stages the
                        # odd-x columns to SBUF, DVE maxes them with even-x.
                        seg = slice(lo // 2, lo // 2 + sz // 2)
                        nc.scalar.copy(todd[:, seg], ps[:, 1:sz:2])
                        nc.vector.tensor_max(t1[:, seg], ps[:, 0:sz:2],
                                             todd[:, seg])
                    # pass2: y-pairs. t1 = (y:48, x2:24)
                    t1v = t1[:].rearrange("p (y xx) -> p y xx", xx=24)
                    t2 = t2p.tile([128, 576], F32, tag="t2", name="t2")
                    t2v = t2[:].rearrange("p (y xx) -> p y xx", xx=24)
                    nc.vector.tensor_max(t2v, t1v[:, 0:48:2, :], t1v[:, 1:48:2, :])
                    # bias+relu -> act1 padded interior (image ci)
                    dst = act1[h][:].rearrange("p (b y x) -> p b y x", y=26, x=26)[
                        :, ci, 1:25, 1:25]
                    nc.scalar.activation(out=dst, in_=t2v, func=RELU,
                                         bias=b1t[:, h:h + 1])

            def conv2_img(g, bb):
                """conv2 for image bb of group g (already in act1)."""
                a1v = [act1[h][:].rearrange("p (b y x) -> p b y x", y=26, x=26)
                       for h in range(2)]
                a2v = act2[:].rearrange("p (b y x) -> p b y x", y=14, x=14)
                if True:
                    for rr in range(2):          # 12-row halves
                        ps = ps2.tile([128, 288], F32, tag="ps2")
                        i = 0
                        for t in range(9):
                            dy, dx = divmod(t, 3)
                            for h in range(2):
                                nc.tensor.matmul(
                                    out=ps[:],
                                    lhsT=r(w2t[:, (t * 2 + h) * 128:(t * 2 + h + 1) * 128]),
                                    rhs=r(a1v[h][:, bb, 12 * rr + dy:12 * rr + dy + 12,
                                                 dx:dx + 24]),
                                    start=(i == 0), stop=(i == 17))
                                i += 1
                        psv = ps[:].rearrange("p (y x) -> p y x", x=24)
                        todd2 = e2p.tile([128, 144], F32, tag="todd2")
                        todd2v = todd2[:].rearrange("p (y x) -> p y x", x=12)
                        nc.scalar.copy(todd2v, psv[:, :, 1:24:2])
                        t1c = e2p.tile([128, 144], F32, tag="t1c2")
                        t1cv = t1c[:].rearrange("p (y x) -> p y x", x=12)
                        nc.vector.tensor_max(t1cv, psv[:, :, 0:24:2], todd2v)
                        t2c = e2p.tile([128, 72], F32, tag="t2c2")
                        t2cv = t2c[:].rearrange("p (y x) -> p y x", x=12)
                        nc.vector.tensor_max(t2cv, t1cv[:, 0:12:2, :], t1cv[:, 1:12:2, :])
                        dst = a2v[:, g * G + bb, 6 * rr + 1:6 * rr + 7, 1:13]
                        nc.scalar.activation(out=dst, in_=t2cv, func=RELU,
                                             bias=b2t[:])

            # ---- group loop: conv1(g) interleaved with conv2(g-1) ----
            prev_g = None
            for g in range(NG):
                xpv = xpad[:].rearrange("p (y x) -> p y x", x=50)
                nc.sync.dma_start(
                    out=xpv[0:G, 1:49, 1:49],
                    in_=x[G * g:G * (g + 1), :].rearrange("b (y x) -> b y x", x=48))
                for ci in range(G):
                    # consume act1 image ci of the previous group BEFORE
                    # conv1 of this group overwrites it (program order is
                    # dependency order for the Tile scheduler).
                    if prev_g is not None:
                        conv2_img(prev_g, ci)
                    imt = cp.tile([9, 2304], F32R, tag="imt", name="imt")
                    for k in range(9):
                        dy, dx = divmod(k, 3)
                        nc.sync.dma_start(
                            out=imt[k:k + 1, :],
                            in_=xpv[ci:ci + 1, dy:dy + 48, dx:dx + 48])
                    conv1_img(g, ci, imt)
                prev_g = g
            for ci in range(G):
                conv2_img(prev_g, ci)

        # ---- conv3 (all groups done; act2 complete) ----
        with contextlib.ExitStack() as cctx:
            ps3 = cctx.enter_context(tc.tile_pool(name="ps3", bufs=3, space="PSUM"))
            psq = cctx.enter_context(tc.tile_pool(name="psq", bufs=1, space="PSUM"))
            pssc = cctx.enter_context(tc.tile_pool(name="pssc", bufs=1, space="PSUM"))
            psT = cctx.enter_context(tc.tile_pool(name="psT", bufs=1, space="PSUM"))

            a2v = act2[:].rearrange("p (b y x) -> p b y x", y=14, x=14)
            o3v = out3[:].rearrange("p (b hw) -> p b hw", hw=36)
            for t in range(32):  # image pairs
                ps = ps3.tile([64, 288], F32, tag="ps3")
                for k in range(9):
                    dy, dx = divmod(k, 3)
                    nc.tensor.matmul(
                        out=ps[:],
                        lhsT=r(w3t[:, 64 * k:64 * (k + 1)]),
                        rhs=r(a2v[:, 2 * t:2 * t + 2, dy:dy + 12, dx:dx + 12]),
                        start=(k == 0), stop=(k == 8))
                psv = ps[:].rearrange("p (b y x) -> p b y x", y=12, x=12)
                todd3 = e3p.tile([64, 144], F32, tag="todd3")
                todd3v = todd3[:].rearrange("p (b y x) -> p b y x", y=12, x=6)
                nc.scalar.copy(todd3v, psv[:, :, :, 1:12:2])
                t1c = e3p.tile([64, 144], F32, tag="t1c3")
                t1cv = t1c[:].rearrange("p (b y x) -> p b y x", y=12, x=6)
                nc.vector.tensor_max(t1cv, psv[:, :, :, 0:12:2], todd3v)
                t2c = e3p.tile([64, 72], F32, tag="t2c3")
                t2cv = t2c[:].rearrange("p (b y x) -> p b y x", y=6, x=6)
                nc.vector.tensor_max(t2cv, t1cv[:, :, 0:12:2, :], t1cv[:, :, 1:12:2, :])
                nc.scalar.activation(
                    out=o3v[:, 2 * t:2 * t + 2, :].rearrange("p b (y x) -> p b y x", x=6),
                    in_=t2cv, func=RELU, bias=b3t[:])

            # ---- q = W_eff @ out4 + b_eff : accumulate over hw ----
            psq_t = psq.tile([64, 64], F32)
            for hw in range(36):
                nc.tensor.matmul(
                    out=psq_t[:],
                    lhsT=r(wet[:, 64 * hw:64 * (hw + 1)]),
                    rhs=r(out3[:, hw:2304:36]),
                    start=(hw == 0), stop=(hw == 35))
            q_sb = ap_pool.tile([64, 64], F32R)
            nc.vector.tensor_scalar_add(q_sb[:], psq_t[:], bet[:])

            # ---- scores: per-image matmuls -> [36, 64] psum ----
            pssc_t = pssc.tile([36, 64], F32)
            for b in range(BPC):
                # fp32r needs even moving-dim; N=1 runs as plain fp32
                nc.tensor.matmul(
                    out=pssc_t[:, b:b + 1],
                    lhsT=out3[:, 36 * b:36 * (b + 1)].bitcast(F32),
                    rhs=q_sb[:, b:b + 1].bitcast(F32),
                    start=True, stop=True)
            sc_sb = ap_pool.tile([36, 64], F32)
            nc.vector.tensor_copy(sc_sb[:], pssc_t[:])
            psT_t = psT.tile([64, 36], F32)
            nc.tensor.transpose(psT_t[:], sc_sb[:], ident[0:36, 0:36])

            # ---- softmax over hw (free dim) ----
            mx = ap_pool.tile([64, 1], F32)
            nc.vector.tensor_reduce(out=mx[:], in_=psT_t[:],
                                    op=mybir.AluOpType.max,
                                    axis=mybir.AxisListType.X)
            nmx = ap_pool.tile([64, 1], F32)
            nc.vector.tensor_scalar_mul(nmx[:], mx[:], -1.0)
            e_t = ap_pool.tile([64, 36], F32)
            nc.scalar.activation(out=e_t[:], in_=psT_t[:], func=EXP, bias=nmx[:])
            z = ap_pool.tile([64, 1], F32)
            nc.vector.tensor_reduce(out=z[:], in_=e_t[:],
                                    op=mybir.AluOpType.add,
                                    axis=mybir.AxisListType.X)
            rz = ap_pool.tile([64, 1], F32)
            nc.vector.reciprocal(rz[:], z[:])
            attn = ap_pool.tile([64, 36], F32R)
            nc.vector.tensor_scalar_mul(attn[:], e_t[:], rz[:])
            if debug:
                nc.sync.dma_start(out=dbg["dbg_act1_0"][:], in_=act1[0][:].bitcast(F32))
                nc.sync.dma_start(out=dbg["dbg_act1_1"][:], in_=act1[1][:].bitcast(F32))
                nc.sync.dma_start(out=dbg["dbg_act2"][:], in_=act2[:].bitcast(F32))
                nc.sync.dma_start(out=dbg["dbg_out3"][:], in_=out3[:].bitcast(F32))
                nc.sync.dma_start(out=dbg["dbg_q"][:], in_=q_sb[:].bitcast(F32))
                nc.sync.dma_start(out=dbg["dbg_attn"][:], in_=attn[:].bitcast(F32))
                nc.sync.dma_start(out=dbg["dbg_sc"][:], in_=sc_sb[:])

        # ---- g_mod + fc3 ----
        with contextlib.ExitStack() as cctx:
            psab = cctx.enter_context(tc.tile_pool(name="psab", bufs=1, space="PSUM"))
            psf = cctx.enter_context(tc.tile_pool(name="psf", bufs=1, space="PSUM"))

            attn_flat = ap_pool.tile([1, 2304], F32R)
            nc.sync.dma_start(out=attn_flat[:], in_=attn[:])
            psab_t = psab.tile([64, 2304], F32)
            for c in range(5):
                lo = 512 * c
                hi = min(lo + 512, 2304)
                nc.tensor.matmul(out=psab_t[:, lo:hi], lhsT=r(ones1[:]),
                                 rhs=r(attn_flat[:, lo:hi]), start=True, stop=True)
            # in-place: out3 is not needed after this product
            nc.vector.tensor_mul(out3[:], out3[:], psab_t[:])
            gT = ap_pool.tile([64, 64], F32R)
            with nc.allow_low_precision(reason="f32r output of attn-weighted sum"):
                nc.vector.tensor_reduce(
                    out=gT[:], in_=out3[:].rearrange("p (b hw) -> p b hw", hw=36),
                    op=mybir.AluOpType.add, axis=mybir.AxisListType.X)

            if debug:
                nc.sync.dma_start(out=dbg["dbg_gT"][:], in_=gT[:].bitcast(F32))
            psf_t = psf.tile([64, 7], F32)
            nc.tensor.matmul(out=psf_t[:], lhsT=gT[:].bitcast(F32),
                             rhs=w3f[:].bitcast(F32), start=True, stop=True)
            out_sb = ap_pool.tile([64, 7], F32)
            nc.vector.tensor_add(out_sb[:], psf_t[:], fc3b_t[:])
            nc.sync.dma_start(out=out[:], in_=out_sb[:])

    _split_excess_waits(nc)
    return nc


def kernel(**inputs):
    from concourse.bass_utils import run_bass_kernel_spmd

    w = _prep_weights({k: np.asarray(v, np.float32) for k, v in inputs.items()
                       if k != 'x'})
    xs = np.asarray(inputs['x'], np.float32).reshape(B_TOTAL, 2304)

    nc = build_program()
    in_maps = []
    for c in range(N_CORES):
        m = {'x': np.ascontiguousarray(xs[BPC * c:BPC * (c + 1)])}
        m.update({k: v for k, v in w.items()})
        in_maps.append(m)
    res = run_bass_kernel_spmd(nc, in_maps, list(range(N_CORES)))
    outs = [res.results[c]['out'] for c in range(N_CORES)]
    return np.concatenate(outs, axis=0).astype(np.float32)


if __name__ == '__main__':
    # quick shape sanity with random data
    rng = np.random.default_rng(0)
    fake = {
        'x': rng.standard_normal((512, 1, 48, 48), dtype=np.float32),
        'conv1_w': rng.standard_normal((256, 1, 3, 3), dtype=np.float32) * 0.05,
        'conv1_b': np.zeros(256, np.float32),
        'bn1_g': np.ones(256, np.float32), 'bn1_b': np.zeros(256, np.float32),
        'bn1_m': np.zeros(256, np.float32), 'bn1_v': np.ones(256, np.float32),
        'conv2_w': rng.standard_normal((128, 256, 3, 3), dtype=np.float32) * 0.05,
        'conv2_b': np.zeros(128, np.float32),
        'bn2_g': np.ones(128, np.float32), 'bn2_b': np.zeros(128, np.float32),
        'bn2_m': np.zeros(128, np.float32), 'bn2_v': np.ones(128, np.float32),
        'conv3_w': rng.standard_normal((64, 128, 3, 3), dtype=np.float32) * 0.05,
        'conv3_b': np.zeros(64, np.float32),
        'bn3_g': np.ones(64, np.float32), 'bn3_b': np.zeros(64, np.float32),
        'bn3_m': np.zeros(64, np.float32), 'bn3_v': np.ones(64, np.float32),
        'fc1_w': rng.standard_normal((512, 2304), dtype=np.float32) * 0.05,
        'fc1_b': np.zeros(512, np.float32),
        'fc2_w': rng.standard_normal((256, 512), dtype=np.float32) * 0.05,
        'fc2_b': np.zeros(256, np.float32),
        'att_w': rng.standard_normal((64, 256), dtype=np.float32) * 0.05,
        'att_b': np.zeros(64, np.float32),
        'fc3_w': rng.standard_normal((7, 64), dtype=np.float32) * 0.05,
        'fc3_b': np.zeros(7, np.float32),
    }
    print(kernel(**fake).shape)


# revision 30
# speedup vs baseline: 1.1312x; 1.1312x over previous
"""Trainium2 Bass kernel for nn_BaselineModel_80796924772520 (dense_cnn).

Self-contained: kernel(**inputs) -> np.ndarray [512, 7] float32.

Strategy: pure data parallelism over 8 NeuronCores (64 images each).
 - BN folded into conv weights/biases on host (eval-mode BN is affine).
 - fc1/fc2/att collapse into one linear map W_eff [64, 2304] on host
   (reference has no nonlinearity between them).
 - All matmuls run as float32r (fast fp32 PE mode, 1 cycle/row at N>=256).
 - conv1 (C_in=1): im2col K=9 built by DMA from a zero-padded x copy.
 - conv2/conv3: 9-tap shifted-window accumulating matmuls over zero-padded
   SBUF activations (no im2col materialization).
 - maxpool: strided tensor_max pairs, pool applied to raw PSUM before
   bias+relu (max commutes with per-channel bias add; relu is monotone).
 - attention: per-image [64x36]^T@[64x1] matmuls -> PE transpose ->
   softmax -> broadcast-matmul with ones -> multiply+segmented reduce.
"""
import sys
if '/opt/trn_rl_repo' not in sys.path:
    sys.path.insert(0, '/opt/trn_rl_repo')

import contextlib
import numpy as np

import concourse.bass as bass
import concourse.mybir as mybir
import concourse.tile as tile
from concourse.masks import make_identity

F32 = mybir.dt.float32
F32R = mybir.dt.float32r
BF16 = mybir.dt.bfloat16
DT_MM = BF16   # matmul-path dtype: BF16 (fast weight load) or F32R
RELU = mybir.ActivationFunctionType.Relu
EXP = mybir.ActivationFunctionType.Exp

N_CORES = 8
B_TOTAL = 512
BPC = B_TOTAL // N_CORES   # 64 images per core
G = 8                      # images per group
NG = BPC // G              # 8 groups
EPS = 1e-5

_MAX_WAITS = 1  # this walrus build supports 1 sync-wait per instruction


def _install_tile_fixups():
    """The nix walrus here allows only ONE sync-wait per instruction; Tile's
    exit drain aggregates one wait per live proc onto a single Drain. Spread
    the waits across spare SP nops emitted just before the drain."""
    if getattr(tile.TileContext, '_drain_patched', False):
        return

    def _patched(self, tick_clock, wait_clock):
        from concourse.vector_clock import ScopedClock
        nc = self.nc
        nops = [nc.sync.nop().ins for _ in range(32)]
        drain_inst = nc.sync.drain()
        wait_clock.add_sem_waits(
            drain_inst.ins, ScopedClock({None: tick_clock.global_clock}))
        si = drain_inst.ins.sync_info
        if si is not None and len(si.on_wait) > _MAX_WAITS:
            waits = list(si.on_wait)
            drain_inst.ins.sync_info = mybir.SyncInfo(
                on_wait=waits[:_MAX_WAITS], on_update=list(si.on_update))
            rest = waits[_MAX_WAITS:]
            for i in range(0, len(rest), _MAX_WAITS):
                nops[i // _MAX_WAITS].sync_info = mybir.SyncInfo(
                    on_wait=rest[i:i + _MAX_WAITS], on_update=[])
        nc.all_engine_barrier()
        popped = nc._tile_sem_poison_stack.pop()
        assert popped is self._sem_poison
        nc.clear_and_free_semaphores(list(self.sems.allocated().values()))
        nc.all_engine_barrier()

    tile.TileContext._drain_and_barrier = _patched
    tile.TileContext._drain_patched = True


def _split_excess_waits(nc):
    """This walrus allows one sync-wait per instruction. Hoist excess waits
    onto same-engine nops inserted immediately before the instruction
    (sequential waits on one engine are equivalent to a combined wait)."""
    idx = 0
    for f in nc.m.functions:
        for b in f.blocks:
            out, changed = [], False
            for ins in b.instructions:
                si = ins.sync_info
                if si is not None and len(si.on_wait) > _MAX_WAITS:
                    waits = list(si.on_wait)
                    extra, keep = waits[:-_MAX_WAITS], waits[-_MAX_WAITS:]
                    for j in range(0, len(extra), _MAX_WAITS):
                        nop = mybir.InstNoOp(name=f"I-wsplit-{idx}")
                        idx += 1
                        nop.engine = ins.engine
                        nop.sync_info = mybir.SyncInfo(
                            on_wait=extra[j:j + _MAX_WAITS], on_update=[])
                        nc.register_instruction(nop, overwrite=True)
                        out.append(nop)
                    ins.sync_info = mybir.SyncInfo(
                        on_wait=keep, on_update=list(si.on_update))
                    changed = True
                out.append(ins)
            if changed:
                b.instructions = out


def _prep_weights(p):
    """Fold BN, collapse FC chain, lay out weights for the device program."""
    def fold(w, b, g, be, m, v):
        inv = (g / np.sqrt(v + EPS)).astype(np.float32)
        wf = (w * inv[:, None, None, None]).astype(np.float32)
        bf = ((b - m) * inv + be).astype(np.float32)
        return wf, bf

    w1, b1 = fold(p['conv1_w'], p['conv1_b'], p['bn1_g'], p['bn1_b'], p['bn1_m'], p['bn1_v'])
    w2, b2 = fold(p['conv2_w'], p['conv2_b'], p['bn2_g'], p['bn2_b'], p['bn2_m'], p['bn2_v'])
    w3, b3 = fold(p['conv3_w'], p['conv3_b'], p['bn3_g'], p['bn3_b'], p['bn3_m'], p['bn3_v'])

    # conv1 lhsT [128, 256]: rows 32q+k (k = 3*ky+kx) = w1[c, 0, ky, kx],
    # replicated into all 4 row-quadrants (enables 32-row PE tiling later).
    W1T = np.zeros((128, 256), np.float32)
    w1f = w1.reshape(256, 9)                      # [c, k]
    for q in range(4):
        W1T[32 * q:32 * q + 9, :] = w1f.T
    # conv2 lhsT [128, 2304]: [p, t*256 + h*128 + m] = w2[m, 128h+p, t]
    W2T = np.ascontiguousarray(
        w2.reshape(128, 2, 128, 9).transpose(2, 3, 1, 0)  # [p, t, h, m]
    ).reshape(128, 2304)
    # conv3 lhsT [128, 576]: [p, t*64 + m] = w3[m, p, t]
    W3T = np.ascontiguousarray(
        w3.reshape(64, 128, 9).transpose(1, 2, 0)).reshape(128, 576)

    # FC chain collapse: q = out4 @ W_eff.T + b_eff
    fc1w, fc2w, attw = p['fc1_w'], p['fc2_w'], p['att_w']
    W_eff = (attw @ fc2w @ fc1w).astype(np.float32)          # [64, 2304]
    b_eff = (attw @ (fc2w @ p['fc1_b'] + p['fc2_b']) + p['att_b']).astype(np.float32)
    # WeT2 [64, 2304]: [c, hw*64 + m] = W_eff[m, c*36 + hw]
    WeT2 = np.ascontiguousarray(
        W_eff.reshape(64, 64, 36).transpose(1, 2, 0)).reshape(64, 2304)

    W3fT = np.ascontiguousarray(p['fc3_w'].T).astype(np.float32)  # [64, 7]
    fc3b_rep = np.broadcast_to(p['fc3_b'], (64, 7)).astype(np.float32).copy()

    b1c = np.ascontiguousarray(b1.reshape(2, 128).T)       # [128, 2]
    b2c = b2.reshape(128, 1).astype(np.float32)
    b3c = b3.reshape(64, 1).astype(np.float32)
    beffc = b_eff.reshape(64, 1).astype(np.float32)

    return dict(W1T=W1T, W2T=W2T, W3T=W3T, WeT2=WeT2, W3fT=W3fT,
                fc3b_rep=fc3b_rep, b1c=b1c, b2c=b2c, b3c=b3c, beffc=beffc,
                Z=np.zeros((1, 12544), np.float32),
                ONES=np.ones((1, 64), np.float32))


def build_program(debug=False):
    """Build the per-core SPMD Bass program. Returns nc."""
    _install_tile_fixups()
    nc = bass.Bass("TRN2", target_bir_lowering=False, debug=False)

    x = nc.declare_dram_parameter("x", [BPC, 2304], DT_MM, isOutput=False)
    W1T = nc.declare_dram_parameter("W1T", [128, 256], DT_MM, isOutput=False)
    W2T = nc.declare_dram_parameter("W2T", [128, 2304], DT_MM, isOutput=False)
    W3T = nc.declare_dram_parameter("W3T", [128, 576], DT_MM, isOutput=False)
    WeT2 = nc.declare_dram_parameter("WeT2", [64, 2304], DT_MM, isOutput=False)
    W3fT = nc.declare_dram_parameter("W3fT", [64, 7], DT_MM, isOutput=False)
    fc3b = nc.declare_dram_parameter("fc3b_rep", [64, 7], F32, isOutput=False)
    b1c = nc.declare_dram_parameter("b1c", [128, 2], F32, isOutput=False)
    b2c = nc.declare_dram_parameter("b2c", [128, 1], F32, isOutput=False)
    b3c = nc.declare_dram_parameter("b3c", [64, 1], F32, isOutput=False)
    beffc = nc.declare_dram_parameter("beffc", [64, 1], F32, isOutput=False)
    Z = nc.declare_dram_parameter("Z", [1, 12544], DT_MM, isOutput=False)
    ONES = nc.declare_dram_parameter("ONES", [1, 64], DT_MM, isOutput=False)
    out = nc.declare_dram_parameter("out", [BPC, 7], F32, isOutput=True)
    dbg = {}
    if debug:
        for nm, shp in [("dbg_act1_0", [128, G * 676]), ("dbg_act1_1", [128, G * 676]),
                        ("dbg_act2", [128, BPC * 196]), ("dbg_out3", [64, BPC * 36]),
                        ("dbg_q", [64, 64]), ("dbg_attn", [64, 36]),
                        ("dbg_gT", [64, 64]), ("dbg_sc", [36, 64])]:
            dbg[nm] = nc.declare_dram_parameter(nm, shp, F32, isOutput=True)

    r = lambda ap: ap                 # tiles are already F32R-typed

    with tile.TileContext(nc) as tc, contextlib.ExitStack() as ctx:
        wp = ctx.enter_context(tc.tile_pool(name="weights", bufs=1))
        ap_pool = ctx.enter_context(tc.tile_pool(name="acts", bufs=1))
        cp = ctx.enter_context(tc.tile_pool(name="im2col", bufs=3))
        t1p = ctx.enter_context(tc.tile_pool(name="t1", bufs=3))
        t2p = ctx.enter_context(tc.tile_pool(name="t2", bufs=3))
        e2p = ctx.enter_context(tc.tile_pool(name="ev2", bufs=4))
        e3p = ctx.enter_context(tc.tile_pool(name="ev3", bufs=3))

        # ---- load weights ----
        w1t = wp.tile([128, 256], DT_MM)
        nc.sync.dma_start(out=w1t[:], in_=W1T[:])
        w2t = wp.tile([128, 2304], DT_MM)
        nc.sync.dma_start(out=w2t[:], in_=W2T[:])
        w3t = wp.tile([128, 576], DT_MM)
        nc.sync.dma_start(out=w3t[:], in_=W3T[:])
        wet = wp.tile([64, 2304], DT_MM)
        nc.sync.dma_start(out=wet[:], in_=WeT2[:])
        w3f = wp.tile([64, 7], DT_MM)
        nc.sync.dma_start(out=w3f[:], in_=W3fT[:])
        fc3b_t = wp.tile([64, 7], F32)
        nc.sync.dma_start(out=fc3b_t[:], in_=fc3b[:])
        b1t = wp.tile([128, 2], F32)
        nc.sync.dma_start(out=b1t[:], in_=b1c[:])
        b2t = wp.tile([128, 1], F32)
        nc.sync.dma_start(out=b2t[:], in_=b2c[:])
        b3t = wp.tile([64, 1], F32)
        nc.sync.dma_start(out=b3t[:], in_=b3c[:])
        bet = wp.tile([64, 1], F32)
        nc.sync.dma_start(out=bet[:], in_=beffc[:])
        ident = wp.tile([128, 128], F32)
        make_identity(nc, ident[:])
        ones1 = wp.tile([1, 64], DT_MM)
        nc.sync.dma_start(out=ones1[:], in_=ONES[:])

        # ---- persistent activation buffers ----
        act1 = [ap_pool.tile([128, G * 676], DT_MM, tag=f"act1_{h}",
                             name=f"act1_{h}") for h in range(2)]
        act2 = ap_pool.tile([128, BPC * 196], DT_MM)
        out3 = ap_pool.tile([64, BPC * 36], DT_MM)
        xpad = ap_pool.tile([8, 2500], DT_MM)
        # DT_MM tensors cannot be Memset on this walrus; zero the pad BORDERS
        # only, via broadcast-DMA from a zeros input (keeps startup off the
        # critical path -- interiors are always fully overwritten).
        # DT_MM tensors cannot be Memset on this walrus; zero-fill padded
        # buffers via broadcast-DMA from a zeros input. xpad first (group 0
        # depends on it); act fills spread across queues to overlap compute.
        nc.gpsimd.dma_start(out=xpad[:], in_=Z[:, :2500].to_broadcast((8, 2500)))
        for h in range(2):
            nc.sync.dma_start(out=act1[h][:],
                              in_=Z[:, :G * 676].to_broadcast((128, G * 676)))
        nc.gpsimd.dma_start(out=act2[:], in_=Z[:].to_broadcast((128, 12544)))

        with contextlib.ExitStack() as cctx:
            ps1 = cctx.enter_context(tc.tile_pool(name="ps1", bufs=2, space="PSUM"))
            ps2 = cctx.enter_context(tc.tile_pool(name="ps2", bufs=4, space="PSUM"))

            def conv1_img(g, ci, imt):
                """conv1 for one image (index ci in group g) + pool + relu."""
                for h in range(2):
                    t1 = t1p.tile([128, 1152], F32, tag="t1", name="t1")
                    todd = t1p.tile([128, 1152], F32, tag="todd", name="todd")
                    # psum tiles: 1024 + 1024 + 256 cols of the 2304-col image
                    for n, (lo, sz) in enumerate(((0, 1024), (1024, 1024), (2048, 256))):
                        ps = ps1.tile([128, 1024], F32, tag="ps1", name="ps1")
                        for j in range(0, sz, 512):
                            w = min(512, sz - j)
                            nc.tensor.matmul(
                                out=ps[:, j:j + w],
                                lhsT=r(w1t[0:9, 128 * h:128 * (h + 1)]),
                                rhs=r(imt[0:9, lo + j:lo + j + w]),
                                start=True, stop=True)
                        # DVE may read only one PSUM operand: ACT stages the
                        # odd-x columns to SBUF, DVE maxes them with even-x.
                        seg = slice(lo // 2, lo // 2 + sz // 2)
                        nc.scalar.copy(todd[:, seg], ps[:, 1:sz:2])
                        nc.vector.tensor_max(t1[:, seg], ps[:, 0:sz:2],
                                             todd[:, seg])
                    # pass2: y-pairs. t1 = (y:48, x2:24)
                    t1v = t1[:].rearrange("p (y xx) -> p y xx", xx=24)
                    t2 = t2p.tile([128, 576], F32, tag="t2", name="t2")
                    t2v = t2[:].rearrange("p (y xx) -> p y xx", xx=24)
                    nc.vector.tensor_max(t2v, t1v[:, 0:48:2, :], t1v[:, 1:48:2, :])
                    # bias+relu -> act1 padded interior (image ci)
                    dst = act1[h][:].rearrange("p (b y x) -> p b y x", y=26, x=26)[
                        :, ci, 1:25, 1:25]
                    nc.scalar.activation(out=dst, in_=t2v, func=RELU,
                                         bias=b1t[:, h:h + 1])

            def conv2_img(g, bb):
                """conv2 for image bb of group g (already in act1)."""
                a1v = [act1[h][:].rearrange("p (b y x) -> p b y x", y=26, x=26)
                       for h in range(2)]
                a2v = act2[:].rearrange("p (b y x) -> p b y x", y=14, x=14)
                if True:
                    for rr in range(2):          # 12-row halves
                        ps = ps2.tile([128, 288], F32, tag="ps2")
                        i = 0
                        for t in range(9):
                            dy, dx = divmod(t, 3)
                            for h in range(2):
                                nc.tensor.matmul(
                                    out=ps[:],
                                    lhsT=r(w2t[:, (t * 2 + h) * 128:(t * 2 + h + 1) * 128]),
                                    rhs=r(a1v[h][:, bb, 12 * rr + dy:12 * rr + dy + 12,
                                                 dx:dx + 24]),
                                    start=(i == 0), stop=(i == 17))
                                i += 1
                        psv = ps[:].rearrange("p (y x) -> p y x", x=24)
                        todd2 = e2p.tile([128, 144], F32, tag="todd2")
                        todd2v = todd2[:].rearrange("p (y x) -> p y x", x=12)
                        nc.scalar.copy(todd2v, psv[:, :, 1:24:2])
                        t1c = e2p.tile([128, 144], F32, tag="t1c2")
                        t1cv = t1c[:].rearrange("p (y x) -> p y x", x=12)
                        nc.vector.tensor_max(t1cv, psv[:, :, 0:24:2], todd2v)
                        t2c = e2p.tile([128, 72], F32, tag="t2c2")
                        t2cv = t2c[:].rearrange("p (y x) -> p y x", x=12)
                        nc.vector.tensor_max(t2cv, t1cv[:, 0:12:2, :], t1cv[:, 1:12:2, :])
                        dst = a2v[:, g * G + bb, 6 * rr + 1:6 * rr + 7, 1:13]
                        nc.scalar.activation(out=dst, in_=t2cv, func=RELU,
                                             bias=b2t[:])

            # ---- group loop: conv1(g) interleaved with conv2(g-1) ----
            prev_g = None
            for g in range(NG):
                xpv = xpad[:].rearrange("p (y x) -> p y x", x=50)
                nc.sync.dma_start(
                    out=xpv[0:G, 1:49, 1:49],
                    in_=x[G * g:G * (g + 1), :].rearrange("b (y x) -> b y x", x=48))
                for ci in range(G):
                    # consume act1 image ci of the previous group BEFORE
                    # conv1 of this group overwrites it (program order is
                    # dependency order for the Tile scheduler).
                    if prev_g is not None:
                        conv2_img(prev_g, ci)
                    imt = cp.tile([9, 2304], DT_MM, tag="imt", name="imt")
                    for k in range(9):
                        dy, dx = divmod(k, 3)
                        eng = (nc.sync, nc.gpsimd)[k % 2]
                        eng.dma_start(
                            out=imt[k:k + 1, :],
                            in_=xpv[ci:ci + 1, dy:dy + 48, dx:dx + 48])
                    conv1_img(g, ci, imt)
                prev_g = g
            for ci in range(G):
                conv2_img(prev_g, ci)

        # ---- conv3 (all groups done; act2 complete) ----
        with contextlib.ExitStack() as cctx:
            ps3 = cctx.enter_context(tc.tile_pool(name="ps3", bufs=3, space="PSUM"))
            psq = cctx.enter_context(tc.tile_pool(name="psq", bufs=1, space="PSUM"))
            pssc = cctx.enter_context(tc.tile_pool(name="pssc", bufs=1, space="PSUM"))
            psT = cctx.enter_context(tc.tile_pool(name="psT", bufs=1, space="PSUM"))

            a2v = act2[:].rearrange("p (b y x) -> p b y x", y=14, x=14)
            o3v = out3[:].rearrange("p (b hw) -> p b hw", hw=36)
            for t in range(32):  # image pairs
                ps = ps3.tile([64, 288], F32, tag="ps3")
                for k in range(9):
                    dy, dx = divmod(k, 3)
                    nc.tensor.matmul(
                        out=ps[:],
                        lhsT=r(w3t[:, 64 * k:64 * (k + 1)]),
                        rhs=r(a2v[:, 2 * t:2 * t + 2, dy:dy + 12, dx:dx + 12]),
                        start=(k == 0), stop=(k == 8))
                psv = ps[:].rearrange("p (b y x) -> p b y x", y=12, x=12)
                todd3 = e3p.tile([64, 144], F32, tag="todd3")
                todd3v = todd3[:].rearrange("p (b y x) -> p b y x", y=12, x=6)
                nc.scalar.copy(todd3v, psv[:, :, :, 1:12:2])
                t1c = e3p.tile([64, 144], F32, tag="t1c3")
                t1cv = t1c[:].rearrange("p (b y x) -> p b y x", y=12, x=6)
                nc.vector.tensor_max(t1cv, psv[:, :, :, 0:12:2], todd3v)
                t2c = e3p.tile([64, 72], F32, tag="t2c3")
                t2cv = t2c[:].rearrange("p (b y x) -> p b y x", y=6, x=6)
                nc.vector.tensor_max(t2cv, t1cv[:, :, 0:12:2, :], t1cv[:, :, 1:12:2, :])
                nc.scalar.activation(
                    out=o3v[:, 2 * t:2 * t + 2, :].rearrange("p b (y x) -> p b y x", x=6),
                    in_=t2cv, func=RELU, bias=b3t[:])

            # ---- q = W_eff @ out4 + b_eff : accumulate over hw ----
            psq_t = psq.tile([64, 64], F32)
            for hw in range(36):
                nc.tensor.matmul(
                    out=psq_t[:],
                    lhsT=r(wet[:, 64 * hw:64 * (hw + 1)]),
                    rhs=r(out3[:, hw:2304:36]),
                    start=(hw == 0), stop=(hw == 35))
            q_sb = ap_pool.tile([64, 64], DT_MM)
            nc.vector.tensor_scalar_add(q_sb[:], psq_t[:], bet[:])

            # ---- scores: per-image matmuls -> [36, 64] psum ----
            pssc_t = pssc.tile([36, 64], F32)
            for b in range(BPC):
                # fp32r needs an even moving-dim; N=1 must run as fp32 then
                cast = (lambda ap: ap.bitcast(F32)) if DT_MM == F32R else (lambda ap: ap)
                nc.tensor.matmul(
                    out=pssc_t[:, b:b + 1],
                    lhsT=cast(out3[:, 36 * b:36 * (b + 1)]),
                    rhs=cast(q_sb[:, b:b + 1]),
                    start=True, stop=True)
            sc_sb = ap_pool.tile([36, 64], F32)
            nc.vector.tensor_copy(sc_sb[:], pssc_t[:])
            psT_t = psT.tile([64, 36], F32)
            nc.tensor.transpose(psT_t[:], sc_sb[:], ident[0:36, 0:36])

            # ---- softmax over hw (free dim) ----
            mx = ap_pool.tile([64, 1], F32)
            nc.vector.tensor_reduce(out=mx[:], in_=psT_t[:],
                                    op=mybir.AluOpType.max,
                                    axis=mybir.AxisListType.X)
            nmx = ap_pool.tile([64, 1], F32)
            nc.vector.tensor_scalar_mul(nmx[:], mx[:], -1.0)
            e_t = ap_pool.tile([64, 36], F32)
            nc.scalar.activation(out=e_t[:], in_=psT_t[:], func=EXP, bias=nmx[:])
            z = ap_pool.tile([64, 1], F32)
            nc.vector.tensor_reduce(out=z[:], in_=e_t[:],
                                    op=mybir.AluOpType.add,
                                    axis=mybir.AxisListType.X)
            rz = ap_pool.tile([64, 1], F32)
            nc.vector.reciprocal(rz[:], z[:])
            attn = ap_pool.tile([64, 36], DT_MM)
            nc.vector.tensor_scalar_mul(attn[:], e_t[:], rz[:])
            if debug:
                nc.gpsimd.dma_start(out=dbg["dbg_act1_0"][:], in_=act1[0][:])
                nc.gpsimd.dma_start(out=dbg["dbg_act1_1"][:], in_=act1[1][:])
                nc.gpsimd.dma_start(out=dbg["dbg_act2"][:], in_=act2[:])
                nc.gpsimd.dma_start(out=dbg["dbg_out3"][:], in_=out3[:])
                nc.gpsimd.dma_start(out=dbg["dbg_q"][:], in_=q_sb[:])
                nc.gpsimd.dma_start(out=dbg["dbg_attn"][:], in_=attn[:])
                nc.sync.dma_start(out=dbg["dbg_sc"][:], in_=sc_sb[:])

        # ---- g_mod + fc3 ----
        with contextlib.ExitStack() as cctx:
            psab = cctx.enter_context(tc.tile_pool(name="psab", bufs=1, space="PSUM"))
            psf = cctx.enter_context(tc.tile_pool(name="psf", bufs=1, space="PSUM"))

            attn_flat = ap_pool.tile([1, 2304], DT_MM)
            nc.sync.dma_start(out=attn_flat[:], in_=attn[:])
            psab_t = psab.tile([64, 2304], F32)
            for c in range(5):
                lo = 512 * c
                hi = min(lo + 512, 2304)
                nc.tensor.matmul(out=psab_t[:, lo:hi], lhsT=r(ones1[:]),
                                 rhs=r(attn_flat[:, lo:hi]), start=True, stop=True)
            # in-place: out3 is not needed after this product
            nc.vector.tensor_mul(out3[:], out3[:], psab_t[:])
            gT = ap_pool.tile([64, 64], DT_MM)
            with nc.allow_low_precision(reason="f32r output of attn-weighted sum"):
                nc.vector.tensor_reduce(
                    out=gT[:], in_=out3[:].rearrange("p (b hw) -> p b hw", hw=36),
                    op=mybir.AluOpType.add, axis=mybir.AxisListType.X)

            if debug:
                nc.gpsimd.dma_start(out=dbg["dbg_gT"][:], in_=gT[:])
            psf_t = psf.tile([64, 7], F32)
            cast = (lambda ap: ap.bitcast(F32)) if DT_MM == F32R else (lambda ap: ap)
            nc.tensor.matmul(out=psf_t[:], lhsT=cast(gT[:]),
                             rhs=cast(w3f[:]), start=True, stop=True)
            out_sb = ap_pool.tile([64, 7], F32)
            nc.vector.tensor_add(out_sb[:], psf_t[:], fc3b_t[:])
            nc.sync.dma_start(out=out[:], in_=out_sb[:])

    _split_excess_waits(nc)
    return nc


def kernel(**inputs):
    from concourse.bass_utils import run_bass_kernel_spmd

    w = _prep_weights({k: np.asarray(v, np.float32) for k, v in inputs.items()
                       if k != 'x'})
    npdt = mybir.dt.np(DT_MM)
    for k in ('W1T', 'W2T', 'W3T', 'WeT2', 'W3fT', 'Z', 'ONES'):
        w[k] = w[k].astype(npdt)
    xs = np.asarray(inputs['x'], np.float32).reshape(B_TOTAL, 2304).astype(npdt)

    nc = build_program()
    in_maps = []
    for c in range(N_CORES):
        m = {'x': np.ascontiguousarray(xs[BPC * c:BPC * (c + 1)])}
        m.update({k: v for k, v in w.items()})
        in_maps.append(m)
    res = run_bass_kernel_spmd(nc, in_maps, list(range(N_CORES)))
    outs = [res.results[c]['out'] for c in range(N_CORES)]
    return np.concatenate(outs, axis=0).astype(np.float32)


if __name__ == '__main__':
    # quick shape sanity with random data
    rng = np.random.default_rng(0)
    fake = {
        'x': rng.standard_normal((512, 1, 48, 48), dtype=np.float32),
        'conv1_w': rng.standard_normal((256, 1, 3, 3), dtype=np.float32) * 0.05,
        'conv1_b': np.zeros(256, np.float32),
        'bn1_g': np.ones(256, np.float32), 'bn1_b': np.zeros(256, np.float32),
        'bn1_m': np.zeros(256, np.float32), 'bn1_v': np.ones(256, np.float32),
        'conv2_w': rng.standard_normal((128, 256, 3, 3), dtype=np.float32) * 0.05,
        'conv2_b': np.zeros(128, np.float32),
        'bn2_g': np.ones(128, np.float32), 'bn2_b': np.zeros(128, np.float32),
        'bn2_m': np.zeros(128, np.float32), 'bn2_v': np.ones(128, np.float32),
        'conv3_w': rng.standard_normal((64, 128, 3, 3), dtype=np.float32) * 0.05,
        'conv3_b': np.zeros(64, np.float32),
        'bn3_g': np.ones(64, np.float32), 'bn3_b': np.zeros(64, np.float32),
        'bn3_m': np.zeros(64, np.float32), 'bn3_v': np.ones(64, np.float32),
        'fc1_w': rng.standard_normal((512, 2304), dtype=np.float32) * 0.05,
        'fc1_b': np.zeros(512, np.float32),
        'fc2_w': rng.standard_normal((256, 512), dtype=np.float32) * 0.05,
        'fc2_b': np.zeros(256, np.float32),
        'att_w': rng.standard_normal((64, 256), dtype=np.float32) * 0.05,
        'att_b': np.zeros(64, np.float32),
        'fc3_w': rng.standard_normal((7, 64), dtype=np.float32) * 0.05,
        'fc3_b': np.zeros(7, np.float32),
    }
    print(kernel(**fake).shape)


# revision 32
# speedup vs baseline: 1.7751x; 1.5693x over previous
"""Trainium2 Bass kernel for nn_BaselineModel_80796924772520 (dense_cnn).

Self-contained: kernel(**inputs) -> np.ndarray [512, 7] float32.

Strategy: pure data parallelism over 8 NeuronCores (64 images each).
 - BN folded into conv weights/biases on host (eval-mode BN is affine).
 - fc1/fc2/att collapse into one linear map W_eff [64, 2304] on host
   (reference has no nonlinearity between them).
 - All matmuls run as float32r (fast fp32 PE mode, 1 cycle/row at N>=256).
 - conv1 (C_in=1): im2col K=9 built by DMA from a zero-padded x copy.
 - conv2/conv3: 9-tap shifted-window accumulating matmuls over zero-padded
   SBUF activations (no im2col materialization).
 - maxpool: strided tensor_max pairs, pool applied to raw PSUM before
   bias+relu (max commutes with per-channel bias add; relu is monotone).
 - attention: per-image [64x36]^T@[64x1] matmuls -> PE transpose ->
   softmax -> broadcast-matmul with ones -> multiply+segmented reduce.
"""
import sys
if '/opt/trn_rl_repo' not in sys.path:
    sys.path.insert(0, '/opt/trn_rl_repo')

import contextlib
import numpy as np

import concourse.bass as bass
import concourse.mybir as mybir
import concourse.tile as tile
from concourse.masks import make_identity

F32 = mybir.dt.float32
F32R = mybir.dt.float32r
BF16 = mybir.dt.bfloat16
DT_MM = BF16   # matmul-path dtype: BF16 (fast weight load) or F32R
RELU = mybir.ActivationFunctionType.Relu
EXP = mybir.ActivationFunctionType.Exp

N_CORES = 8
B_TOTAL = 512
BPC = B_TOTAL // N_CORES   # 64 images per core
G = 8                      # images per group
NG = BPC // G              # 8 groups
EPS = 1e-5

_MAX_WAITS = 1  # this walrus build supports 1 sync-wait per instruction


def _install_tile_fixups():
    """The nix walrus here allows only ONE sync-wait per instruction; Tile's
    exit drain aggregates one wait per live proc onto a single Drain. Spread
    the waits across spare SP nops emitted just before the drain."""
    if getattr(tile.TileContext, '_drain_patched', False):
        return

    def _patched(self, tick_clock, wait_clock):
        from concourse.vector_clock import ScopedClock
        nc = self.nc
        nops = [nc.sync.nop().ins for _ in range(32)]
        drain_inst = nc.sync.drain()
        wait_clock.add_sem_waits(
            drain_inst.ins, ScopedClock({None: tick_clock.global_clock}))
        si = drain_inst.ins.sync_info
        if si is not None and len(si.on_wait) > _MAX_WAITS:
            waits = list(si.on_wait)
            drain_inst.ins.sync_info = mybir.SyncInfo(
                on_wait=waits[:_MAX_WAITS], on_update=list(si.on_update))
            rest = waits[_MAX_WAITS:]
            for i in range(0, len(rest), _MAX_WAITS):
                nops[i // _MAX_WAITS].sync_info = mybir.SyncInfo(
                    on_wait=rest[i:i + _MAX_WAITS], on_update=[])
        nc.all_engine_barrier()
        popped = nc._tile_sem_poison_stack.pop()
        assert popped is self._sem_poison
        nc.clear_and_free_semaphores(list(self.sems.allocated().values()))
        nc.all_engine_barrier()

    tile.TileContext._drain_and_barrier = _patched
    tile.TileContext._drain_patched = True


def _split_excess_waits(nc):
    """This walrus allows one sync-wait per instruction. Hoist excess waits
    onto same-engine nops inserted immediately before the instruction
    (sequential waits on one engine are equivalent to a combined wait)."""
    idx = 0
    for f in nc.m.functions:
        for b in f.blocks:
            out, changed = [], False
            for ins in b.instructions:
                si = ins.sync_info
                if si is not None and len(si.on_wait) > _MAX_WAITS:
                    waits = list(si.on_wait)
                    extra, keep = waits[:-_MAX_WAITS], waits[-_MAX_WAITS:]
                    for j in range(0, len(extra), _MAX_WAITS):
                        nop = mybir.InstNoOp(name=f"I-wsplit-{idx}")
                        idx += 1
                        nop.engine = ins.engine
                        nop.sync_info = mybir.SyncInfo(
                            on_wait=extra[j:j + _MAX_WAITS], on_update=[])
                        nc.register_instruction(nop, overwrite=True)
                        out.append(nop)
                    ins.sync_info = mybir.SyncInfo(
                        on_wait=keep, on_update=list(si.on_update))
                    changed = True
                out.append(ins)
            if changed:
                b.instructions = out


def _prep_weights(p):
    """Fold BN, collapse FC chain, lay out weights for the device program."""
    def fold(w, b, g, be, m, v):
        inv = (g / np.sqrt(v + EPS)).astype(np.float32)
        wf = (w * inv[:, None, None, None]).astype(np.float32)
        bf = ((b - m) * inv + be).astype(np.float32)
        return wf, bf

    w1, b1 = fold(p['conv1_w'], p['conv1_b'], p['bn1_g'], p['bn1_b'], p['bn1_m'], p['bn1_v'])
    w2, b2 = fold(p['conv2_w'], p['conv2_b'], p['bn2_g'], p['bn2_b'], p['bn2_m'], p['bn2_v'])
    w3, b3 = fold(p['conv3_w'], p['conv3_b'], p['bn3_g'], p['bn3_b'], p['bn3_m'], p['bn3_v'])

    # conv1 lhsT [128, 256]: rows 32q+k (k = 3*ky+kx) = w1[c, 0, ky, kx],
    # replicated into all 4 row-quadrants (enables 32-row PE tiling later).
    W1T = np.zeros((128, 256), np.float32)
    w1f = w1.reshape(256, 9)                      # [c, k]
    for q in range(4):
        W1T[32 * q:32 * q + 9, :] = w1f.T
    # conv2 lhsT [128, 2304]: [p, t*256 + h*128 + m] = w2[m, 128h+p, t]
    W2T = np.ascontiguousarray(
        w2.reshape(128, 2, 128, 9).transpose(2, 3, 1, 0)  # [p, t, h, m]
    ).reshape(128, 2304)
    # conv3 lhsT [128, 576]: [p, t*64 + m] = w3[m, p, t]
    W3T = np.ascontiguousarray(
        w3.reshape(64, 128, 9).transpose(1, 2, 0)).reshape(128, 576)

    # FC chain collapse: q = out4 @ W_eff.T + b_eff
    fc1w, fc2w, attw = p['fc1_w'], p['fc2_w'], p['att_w']
    W_eff = (attw @ fc2w @ fc1w).astype(np.float32)          # [64, 2304]
    b_eff = (attw @ (fc2w @ p['fc1_b'] + p['fc2_b']) + p['att_b']).astype(np.float32)
    # WeT2 [64, 2304]: [c, hw*64 + m] = W_eff[m, c*36 + hw]
    WeT2 = np.ascontiguousarray(
        W_eff.reshape(64, 64, 36).transpose(1, 2, 0)).reshape(64, 2304)

    W3fT = np.ascontiguousarray(p['fc3_w'].T).astype(np.float32)  # [64, 7]
    fc3b_rep = np.broadcast_to(p['fc3_b'], (64, 7)).astype(np.float32).copy()

    b1c = np.ascontiguousarray(b1.reshape(2, 128).T)       # [128, 2]
    b2c = b2.reshape(128, 1).astype(np.float32)
    b3c = b3.reshape(64, 1).astype(np.float32)
    beffc = b_eff.reshape(64, 1).astype(np.float32)

    return dict(W1T=W1T, W2T=W2T, W3T=W3T, WeT2=WeT2, W3fT=W3fT,
                fc3b_rep=fc3b_rep, b1c=b1c, b2c=b2c, b3c=b3c, beffc=beffc,
                Z=np.zeros((1, 12544), np.float32),
                ONES=np.ones((1, 64), np.float32))


def build_program(debug=False):
    """Build the per-core SPMD Bass program. Returns nc."""
    _install_tile_fixups()
    nc = bass.Bass("TRN2", target_bir_lowering=False, debug=False)

    x = nc.declare_dram_parameter("x", [BPC, 2304], DT_MM, isOutput=False)
    W1T = nc.declare_dram_parameter("W1T", [128, 256], DT_MM, isOutput=False)
    W2T = nc.declare_dram_parameter("W2T", [128, 2304], DT_MM, isOutput=False)
    W3T = nc.declare_dram_parameter("W3T", [128, 576], DT_MM, isOutput=False)
    WeT2 = nc.declare_dram_parameter("WeT2", [64, 2304], DT_MM, isOutput=False)
    W3fT = nc.declare_dram_parameter("W3fT", [64, 7], DT_MM, isOutput=False)
    fc3b = nc.declare_dram_parameter("fc3b_rep", [64, 7], F32, isOutput=False)
    b1c = nc.declare_dram_parameter("b1c", [128, 2], F32, isOutput=False)
    b2c = nc.declare_dram_parameter("b2c", [128, 1], F32, isOutput=False)
    b3c = nc.declare_dram_parameter("b3c", [64, 1], F32, isOutput=False)
    beffc = nc.declare_dram_parameter("beffc", [64, 1], F32, isOutput=False)
    Z = nc.declare_dram_parameter("Z", [1, 12544], DT_MM, isOutput=False)
    ONES = nc.declare_dram_parameter("ONES", [1, 64], DT_MM, isOutput=False)
    out = nc.declare_dram_parameter("out", [BPC, 7], F32, isOutput=True)
    dbg = {}
    if debug:
        for nm, shp in [("dbg_act1_0", [128, G * 676]), ("dbg_act1_1", [128, G * 676]),
                        ("dbg_act2", [128, BPC * 196]), ("dbg_out3", [64, BPC * 36]),
                        ("dbg_q", [64, 64]), ("dbg_attn", [64, 36]),
                        ("dbg_gT", [64, 64]), ("dbg_sc", [36, 64])]:
            dbg[nm] = nc.declare_dram_parameter(nm, shp, F32, isOutput=True)

    r = lambda ap: ap                 # tiles are already F32R-typed

    with tile.TileContext(nc) as tc, contextlib.ExitStack() as ctx:
        wp = ctx.enter_context(tc.tile_pool(name="weights", bufs=1))
        ap_pool = ctx.enter_context(tc.tile_pool(name="acts", bufs=1))
        cp = ctx.enter_context(tc.tile_pool(name="im2col", bufs=2))
        t1p = ctx.enter_context(tc.tile_pool(name="t1", bufs=3))
        t2p = ctx.enter_context(tc.tile_pool(name="t2", bufs=3))
        e2p = ctx.enter_context(tc.tile_pool(name="ev2", bufs=4))
        e3p = ctx.enter_context(tc.tile_pool(name="ev3", bufs=3))

        # ---- load weights ----
        w1t = wp.tile([128, 256], DT_MM)
        nc.sync.dma_start(out=w1t[:], in_=W1T[:])
        w2t = wp.tile([128, 2304], DT_MM)
        nc.sync.dma_start(out=w2t[:], in_=W2T[:])
        w3t = wp.tile([128, 576], DT_MM)
        nc.sync.dma_start(out=w3t[:], in_=W3T[:])
        wet = wp.tile([64, 2304], DT_MM)
        nc.sync.dma_start(out=wet[:], in_=WeT2[:])
        w3f = wp.tile([64, 7], DT_MM)
        nc.sync.dma_start(out=w3f[:], in_=W3fT[:])
        fc3b_t = wp.tile([64, 7], F32)
        nc.sync.dma_start(out=fc3b_t[:], in_=fc3b[:])
        b1t = wp.tile([128, 2], F32)
        nc.sync.dma_start(out=b1t[:], in_=b1c[:])
        b2t = wp.tile([128, 1], F32)
        nc.sync.dma_start(out=b2t[:], in_=b2c[:])
        b3t = wp.tile([64, 1], F32)
        nc.sync.dma_start(out=b3t[:], in_=b3c[:])
        bet = wp.tile([64, 1], F32)
        nc.sync.dma_start(out=bet[:], in_=beffc[:])
        ident = wp.tile([128, 128], F32)
        make_identity(nc, ident[:])
        ones1 = wp.tile([1, 64], DT_MM)
        nc.sync.dma_start(out=ones1[:], in_=ONES[:])

        # ---- persistent activation buffers ----
        act1 = [ap_pool.tile([128, G * 676], DT_MM, tag=f"act1_{h}",
                             name=f"act1_{h}") for h in range(2)]
        act2 = ap_pool.tile([128, BPC * 196], DT_MM)
        out3 = ap_pool.tile([64, BPC * 36], DT_MM)
        xpad = ap_pool.tile([8, 2500], DT_MM)
        # DT_MM tensors cannot be Memset on this walrus; zero the pad BORDERS
        # only, via broadcast-DMA from a zeros input (keeps startup off the
        # critical path -- interiors are always fully overwritten).
        # DT_MM tensors cannot be Memset on this walrus; zero-fill padded
        # buffers via broadcast-DMA from a zeros input. xpad first (group 0
        # depends on it); act fills spread across queues to overlap compute.
        nc.gpsimd.dma_start(out=xpad[:], in_=Z[:, :2500].to_broadcast((8, 2500)))
        for h in range(2):
            nc.sync.dma_start(out=act1[h][:],
                              in_=Z[:, :G * 676].to_broadcast((128, G * 676)))
        nc.gpsimd.dma_start(out=act2[:], in_=Z[:].to_broadcast((128, 12544)))

        with contextlib.ExitStack() as cctx:
            ps1 = cctx.enter_context(tc.tile_pool(name="ps1", bufs=2, space="PSUM"))
            ps2 = cctx.enter_context(tc.tile_pool(name="ps2", bufs=4, space="PSUM"))

            def conv1_img(g, ci, imt):
                """conv1 for one image (index ci in group g) + pool + relu."""
                for h in range(2):
                    t1 = t1p.tile([128, 1152], DT_MM, tag="t1", name="t1")
                    todd = t1p.tile([128, 1152], DT_MM, tag="todd", name="todd")
                    # psum tiles: 1024 + 1024 + 256 cols of the 2304-col image
                    for n, (lo, sz) in enumerate(((0, 1024), (1024, 1024), (2048, 256))):
                        ps = ps1.tile([128, 1024], F32, tag="ps1", name="ps1")
                        for j in range(0, sz, 512):
                            w = min(512, sz - j)
                            nc.tensor.matmul(
                                out=ps[:, j:j + w],
                                lhsT=r(w1t[0:9, 128 * h:128 * (h + 1)]),
                                rhs=r(imt[:, lo + j:lo + j + w]),
                                start=True, stop=True)
                        # DVE may read only one PSUM operand: ACT stages the
                        # odd-x columns to SBUF, DVE maxes them with even-x.
                        seg = slice(lo // 2, lo // 2 + sz // 2)
                        nc.scalar.copy(todd[:, seg], ps[:, 1:sz:2])
                        nc.vector.tensor_max(t1[:, seg], ps[:, 0:sz:2],
                                             todd[:, seg])
                    # pass2: y-pairs. t1 = (y:48, x2:24)
                    t1v = t1[:].rearrange("p (y xx) -> p y xx", xx=24)
                    t2 = t2p.tile([128, 576], DT_MM, tag="t2", name="t2")
                    t2v = t2[:].rearrange("p (y xx) -> p y xx", xx=24)
                    nc.vector.tensor_max(t2v, t1v[:, 0:48:2, :], t1v[:, 1:48:2, :])
                    # bias+relu -> act1 padded interior (image ci)
                    dst = act1[h][:].rearrange("p (b y x) -> p b y x", y=26, x=26)[
                        :, ci, 1:25, 1:25]
                    nc.scalar.activation(out=dst, in_=t2v, func=RELU,
                                         bias=b1t[:, h:h + 1])

            def conv2_img(g, bb):
                """conv2 for image bb of group g (already in act1)."""
                a1v = [act1[h][:].rearrange("p (b y x) -> p b y x", y=26, x=26)
                       for h in range(2)]
                a2v = act2[:].rearrange("p (b y x) -> p b y x", y=14, x=14)
                if True:
                    for rr in range(2):          # 12-row halves
                        ps = ps2.tile([128, 288], F32, tag="ps2")
                        i = 0
                        for t in range(9):
                            dy, dx = divmod(t, 3)
                            for h in range(2):
                                nc.tensor.matmul(
                                    out=ps[:],
                                    lhsT=r(w2t[:, (t * 2 + h) * 128:(t * 2 + h + 1) * 128]),
                                    rhs=r(a1v[h][:, bb, 12 * rr + dy:12 * rr + dy + 12,
                                                 dx:dx + 24]),
                                    start=(i == 0), stop=(i == 17))
                                i += 1
                        psv = ps[:].rearrange("p (y x) -> p y x", x=24)
                        todd2 = e2p.tile([128, 144], DT_MM, tag="todd2")
                        todd2v = todd2[:].rearrange("p (y x) -> p y x", x=12)
                        nc.scalar.copy(todd2v, psv[:, :, 1:24:2])
                        t1c = e2p.tile([128, 144], DT_MM, tag="t1c2")
                        t1cv = t1c[:].rearrange("p (y x) -> p y x", x=12)
                        nc.vector.tensor_max(t1cv, psv[:, :, 0:24:2], todd2v)
                        t2c = e2p.tile([128, 72], DT_MM, tag="t2c2")
                        t2cv = t2c[:].rearrange("p (y x) -> p y x", x=12)
                        nc.vector.tensor_max(t2cv, t1cv[:, 0:12:2, :], t1cv[:, 1:12:2, :])
                        dst = a2v[:, g * G + bb, 6 * rr + 1:6 * rr + 7, 1:13]
                        nc.scalar.activation(out=dst, in_=t2cv, func=RELU,
                                             bias=b2t[:])

            # ---- group loop: conv1(g) interleaved with conv2(g-1) ----
            prev_g = None
            for g in range(NG):
                xpv = xpad[:].rearrange("p (y x) -> p y x", x=50)
                nc.sync.dma_start(
                    out=xpv[0:G, 1:49, 1:49],
                    in_=x[G * g:G * (g + 1), :].rearrange("b (y x) -> b y x", x=48))
                imt = cp.tile([9, G * 2304], DT_MM, tag="imt", name="imt")
                for k in range(9):
                    dy, dx = divmod(k, 3)
                    eng = (nc.sync, nc.gpsimd)[k % 2]
                    eng.dma_start(
                        out=imt[k:k + 1, :],
                        in_=xpv[0:G, dy:dy + 48, dx:dx + 48])
                for ci in range(G):
                    # consume act1 image ci of the previous group BEFORE
                    # conv1 of this group overwrites it (program order is
                    # dependency order for the Tile scheduler).
                    if prev_g is not None:
                        conv2_img(prev_g, ci)
                    conv1_img(g, ci, imt[0:9, 2304 * ci:2304 * (ci + 1)])
                prev_g = g
            for ci in range(G):
                conv2_img(prev_g, ci)

        # ---- conv3 (all groups done; act2 complete) ----
        with contextlib.ExitStack() as cctx:
            ps3 = cctx.enter_context(tc.tile_pool(name="ps3", bufs=3, space="PSUM"))
            psq = cctx.enter_context(tc.tile_pool(name="psq", bufs=1, space="PSUM"))
            pssc = cctx.enter_context(tc.tile_pool(name="pssc", bufs=1, space="PSUM"))
            psT = cctx.enter_context(tc.tile_pool(name="psT", bufs=1, space="PSUM"))

            a2v = act2[:].rearrange("p (b y x) -> p b y x", y=14, x=14)
            o3v = out3[:].rearrange("p (b hw) -> p b hw", hw=36)
            for t in range(32):  # image pairs
                ps = ps3.tile([64, 288], F32, tag="ps3")
                for k in range(9):
                    dy, dx = divmod(k, 3)
                    nc.tensor.matmul(
                        out=ps[:],
                        lhsT=r(w3t[:, 64 * k:64 * (k + 1)]),
                        rhs=r(a2v[:, 2 * t:2 * t + 2, dy:dy + 12, dx:dx + 12]),
                        start=(k == 0), stop=(k == 8))
                psv = ps[:].rearrange("p (b y x) -> p b y x", y=12, x=12)
                todd3 = e3p.tile([64, 144], DT_MM, tag="todd3")
                todd3v = todd3[:].rearrange("p (b y x) -> p b y x", y=12, x=6)
                nc.scalar.copy(todd3v, psv[:, :, :, 1:12:2])
                t1c = e3p.tile([64, 144], DT_MM, tag="t1c3")
                t1cv = t1c[:].rearrange("p (b y x) -> p b y x", y=12, x=6)
                nc.vector.tensor_max(t1cv, psv[:, :, :, 0:12:2], todd3v)
                t2c = e3p.tile([64, 72], DT_MM, tag="t2c3")
                t2cv = t2c[:].rearrange("p (b y x) -> p b y x", y=6, x=6)
                nc.vector.tensor_max(t2cv, t1cv[:, :, 0:12:2, :], t1cv[:, :, 1:12:2, :])
                nc.scalar.activation(
                    out=o3v[:, 2 * t:2 * t + 2, :].rearrange("p b (y x) -> p b y x", x=6),
                    in_=t2cv, func=RELU, bias=b3t[:])

            # ---- q = W_eff @ out4 + b_eff : accumulate over hw ----
            psq_t = psq.tile([64, 64], F32)
            for hw in range(36):
                nc.tensor.matmul(
                    out=psq_t[:],
                    lhsT=r(wet[:, 64 * hw:64 * (hw + 1)]),
                    rhs=r(out3[:, hw:2304:36]),
                    start=(hw == 0), stop=(hw == 35))
            q_sb = ap_pool.tile([64, 64], DT_MM)
            nc.vector.tensor_scalar_add(q_sb[:], psq_t[:], bet[:])

            # ---- scores: per-image matmuls -> [36, 64] psum ----
            pssc_t = pssc.tile([36, 64], F32)
            for b in range(BPC):
                # fp32r needs an even moving-dim; N=1 must run as fp32 then
                cast = (lambda ap: ap.bitcast(F32)) if DT_MM == F32R else (lambda ap: ap)
                nc.tensor.matmul(
                    out=pssc_t[:, b:b + 1],
                    lhsT=cast(out3[:, 36 * b:36 * (b + 1)]),
                    rhs=cast(q_sb[:, b:b + 1]),
                    start=True, stop=True)
            sc_sb = ap_pool.tile([36, 64], F32)
            nc.vector.tensor_copy(sc_sb[:], pssc_t[:])
            psT_t = psT.tile([64, 36], F32)
            nc.tensor.transpose(psT_t[:], sc_sb[:], ident[0:36, 0:36])

            # ---- softmax over hw (free dim) ----
            mx = ap_pool.tile([64, 1], F32)
            nc.vector.tensor_reduce(out=mx[:], in_=psT_t[:],
                                    op=mybir.AluOpType.max,
                                    axis=mybir.AxisListType.X)
            nmx = ap_pool.tile([64, 1], F32)
            nc.vector.tensor_scalar_mul(nmx[:], mx[:], -1.0)
            e_t = ap_pool.tile([64, 36], F32)
            nc.scalar.activation(out=e_t[:], in_=psT_t[:], func=EXP, bias=nmx[:])
            z = ap_pool.tile([64, 1], F32)
            nc.vector.tensor_reduce(out=z[:], in_=e_t[:],
                                    op=mybir.AluOpType.add,
                                    axis=mybir.AxisListType.X)
            rz = ap_pool.tile([64, 1], F32)
            nc.vector.reciprocal(rz[:], z[:])
            attn = ap_pool.tile([64, 36], DT_MM)
            nc.vector.tensor_scalar_mul(attn[:], e_t[:], rz[:])
            if debug:
                nc.gpsimd.dma_start(out=dbg["dbg_act1_0"][:], in_=act1[0][:])
                nc.gpsimd.dma_start(out=dbg["dbg_act1_1"][:], in_=act1[1][:])
                nc.gpsimd.dma_start(out=dbg["dbg_act2"][:], in_=act2[:])
                nc.gpsimd.dma_start(out=dbg["dbg_out3"][:], in_=out3[:])
                nc.gpsimd.dma_start(out=dbg["dbg_q"][:], in_=q_sb[:])
                nc.gpsimd.dma_start(out=dbg["dbg_attn"][:], in_=attn[:])
                nc.sync.dma_start(out=dbg["dbg_sc"][:], in_=sc_sb[:])

        # ---- g_mod + fc3 ----
        with contextlib.ExitStack() as cctx:
            psab = cctx.enter_context(tc.tile_pool(name="psab", bufs=1, space="PSUM"))
            psf = cctx.enter_context(tc.tile_pool(name="psf", bufs=1, space="PSUM"))

            attn_flat = ap_pool.tile([1, 2304], DT_MM)
            nc.sync.dma_start(out=attn_flat[:], in_=attn[:])
            psab_t = psab.tile([64, 2304], F32)
            for c in range(5):
                lo = 512 * c
                hi = min(lo + 512, 2304)
                nc.tensor.matmul(out=psab_t[:, lo:hi], lhsT=r(ones1[:]),
                                 rhs=r(attn_flat[:, lo:hi]), start=True, stop=True)
            # in-place: out3 is not needed after this product
            nc.vector.tensor_mul(out3[:], out3[:], psab_t[:])
            gT = ap_pool.tile([64, 64], DT_MM)
            with nc.allow_low_precision(reason="f32r output of attn-weighted sum"):
                nc.vector.tensor_reduce(
                    out=gT[:], in_=out3[:].rearrange("p (b hw) -> p b hw", hw=36),
                    op=mybir.AluOpType.add, axis=mybir.AxisListType.X)

            if debug:
                nc.gpsimd.dma_start(out=dbg["dbg_gT"][:], in_=gT[:])
            psf_t = psf.tile([64, 7], F32)
            cast = (lambda ap: ap.bitcast(F32)) if DT_MM == F32R else (lambda ap: ap)
            nc.tensor.matmul(out=psf_t[:], lhsT=cast(gT[:]),
                             rhs=cast(w3f[:]), start=True, stop=True)
            out_sb = ap_pool.tile([64, 7], F32)
            nc.vector.tensor_add(out_sb[:], psf_t[:], fc3b_t[:])
            nc.sync.dma_start(out=out[:], in_=out_sb[:])

    _split_excess_waits(nc)
    return nc


def kernel(**inputs):
    from concourse.bass_utils import run_bass_kernel_spmd

    w = _prep_weights({k: np.asarray(v, np.float32) for k, v in inputs.items()
                       if k != 'x'})
    npdt = mybir.dt.np(DT_MM)
    for k in ('W1T', 'W2T', 'W3T', 'WeT2', 'W3fT', 'Z', 'ONES'):
        w[k] = w[k].astype(npdt)
    xs = np.asarray(inputs['x'], np.float32).reshape(B_TOTAL, 2304).astype(npdt)

    nc = build_program()
    in_maps = []
    for c in range(N_CORES):
        m = {'x': np.ascontiguousarray(xs[BPC * c:BPC * (c + 1)])}
        m.update({k: v for k, v in w.items()})
        in_maps.append(m)
    res = run_bass_kernel_spmd(nc, in_maps, list(range(N_CORES)))
    outs = [res.results[c]['out'] for c in range(N_CORES)]
    return np.concatenate(outs, axis=0).astype(np.float32)


if __name__ == '__main__':
    # quick shape sanity with random data
    rng = np.random.default_rng(0)
    fake = {
        'x': rng.standard_normal((512, 1, 48, 48), dtype=np.float32),
        'conv1_w': rng.standard_normal((256, 1, 3, 3), dtype=np.float32) * 0.05,
        'conv1_b': np.zeros(256, np.float32),
        'bn1_g': np.ones(256, np.float32), 'bn1_b': np.zeros(256, np.float32),
        'bn1_m': np.zeros(256, np.float32), 'bn1_v': np.ones(256, np.float32),
        'conv2_w': rng.standard_normal((128, 256, 3, 3), dtype=np.float32) * 0.05,
        'conv2_b': np.zeros(128, np.float32),
        'bn2_g': np.ones(128, np.float32), 'bn2_b': np.zeros(128, np.float32),
        'bn2_m': np.zeros(128, np.float32), 'bn2_v': np.ones(128, np.float32),
        'conv3_w': rng.standard_normal((64, 128, 3, 3), dtype=np.float32) * 0.05,
        'conv3_b': np.zeros(64, np.float32),
        'bn3_g': np.ones(64, np.float32), 'bn3_b': np.zeros(64, np.float32),
        'bn3_m': np.zeros(64, np.float32), 'bn3_v': np.ones(64, np.float32),
        'fc1_w': rng.standard_normal((512, 2304), dtype=np.float32) * 0.05,
        'fc1_b': np.zeros(512, np.float32),
        'fc2_w': rng.standard_normal((256, 512), dtype=np.float32) * 0.05,
        'fc2_b': np.zeros(256, np.float32),
        'att_w': rng.standard_normal((64, 256), dtype=np.float32) * 0.05,
        'att_b': np.zeros(64, np.float32),
        'fc3_w': rng.standard_normal((7, 64), dtype=np.float32) * 0.05,
        'fc3_b': np.zeros(7, np.float32),
    }
    print(kernel(**fake).shape)
